# revision 67
# baseline (speedup 1.0000x reference)
"""BigBird block-sparse attention on 8 Trainium2 NeuronCores (v2).

Sharding: core c handles batch b = c // 4 and 3 heads starting at 3 * (c % 4).
Each core computes its partial output projection OUT_c = sum_h ctx_h @ WoT_h
(fp16, [4096, 768]); the host sums the 4 partials per batch and adds the
bias terms (Wo_b plus the Wv bias pushed through the output projection).

v2 design (vs v1): scores are computed directly in transposed layout
(scoresT [keys, queries]) so no PE transposes of probabilities are needed.
Key blocks are processed in PAIRS stacked on 128 PSUM partitions, halving
the streamed columns. V is kept in natural layout with an interleaved ones
column per head ([h0|1|h1|1|h2|1] = 195 cols/chunk); the ctx matmul
lhs [128 keys, 65] then accumulates the softmax denominator as output
row 64 for free. Normalization happens per group of 8 query blocks:
reciprocal of the denominator row, broadcast via a C=1 PE matmul
(ones[1,64] x r[1,512]), then one DVE multiply producing ctxT directly.
Global key blocks {0,63} are scored against all queries in one 8-matmul
sweep per head (expTg), reused as the 4th ctx chunk of every query block.
Global query rows (blocks 0/63) use the same scoresT machinery with the
natural V chunks. Output projection is interleaved with the sparse loop
and written as fp16 to halve the output DMA.
"""

import os

import numpy as np
import ml_dtypes

B, S, HID = 2, 4096, 768
NH, HD = 12, 64
BS = 64
NB = S // BS            # 64 blocks
NR = 3
NCORES = 8
HPC = 3                 # heads per core
NCHUNK = S // 128       # 32 s-chunks of 128
NK6 = HID // 128        # 6 hid-chunks
VW = 3 * 65             # Vn cols per chunk: [h0(64)|1][h1(64)|1][h2(64)|1]

BF16 = ml_dtypes.bfloat16

_cache = {}
_last_in_maps = None


def _pair4(vals):
    """Pair up 4 key-block ids, avoiding equal pairs when possible.
    Returns two (ja, jb) pairs with ja <= jb."""
    s = sorted(vals)
    if s[0] == s[1] and s[1] != s[2]:
        return [(s[0], s[2]), (s[1], s[3])]
    if s[2] == s[3] and s[1] != s[2]:
        return [(s[0], s[2]), (s[1], s[3])]
    return [(s[0], s[1]), (s[2], s[3])]


def _build(idx, upto="G"):
    """Build the SPMD Bass program. idx: tuple of NB tuples of K key-block
    ids (window 3, global 2, rand 3 — only rand entries [5:8] are used;
    window/global are recomputed here).

    upto: last stage to include ("C", "DG", "D", "E", "G") for bisection.
    """
    import concourse.mybir as mybir
    from concourse import bacc
    from concourse.tile import TileContext
    from bass_rust import AP

    dt = mybir.dt
    order = {"C": 0, "DG": 1, "D": 2, "E": 3, "G": 4}
    lvl = order[upto]
    dump = os.environ.get("KDUMP", "0") == "1"
    nc = bacc.Bacc()

    # ---- DRAM tensors ----
    # XT is X pre-transposed on the host: [HID, S]. Plain row-major DMA
    # (8KB/partition lines) replaces the serialized dma_start_transpose.
    XT_d = nc.dram_tensor("XT", [HID, S], dt.bfloat16, kind="ExternalInput")
    WQT_d = nc.dram_tensor("WQT", [HID, 128], dt.bfloat16, kind="ExternalInput")
    WKT_d = nc.dram_tensor("WKT", [HID, 128], dt.bfloat16, kind="ExternalInput")
    WQK2_d = nc.dram_tensor("WQK2", [HID, 128], dt.bfloat16, kind="ExternalInput")
    WVT_d = nc.dram_tensor("WVT", [HID, 192], dt.bfloat16, kind="ExternalInput")
    WOT_d = nc.dram_tensor("WOT", [192, HID], dt.bfloat16, kind="ExternalInput")
    BQ_d = nc.dram_tensor("BQ", [128, 1], dt.float32, kind="ExternalInput")
    BK_d = nc.dram_tensor("BK", [128, 1], dt.float32, kind="ExternalInput")
    BQK2_d = nc.dram_tensor("BQK2", [128, 1], dt.float32, kind="ExternalInput")
    OUT_d = nc.dram_tensor("OUT", [S, HID], dt.float16, kind="ExternalOutput")
    dmp = {}
    if dump:
        for nm, shp in (
            ("DQT01", [128, S]), ("DKT01", [128, S]), ("DQK2", [128, S]),
            ("DK2LO", [64, S]), ("DVN", [128, NCHUNK * VW]),
            ("DCTXT01", [128, S]), ("DCTXT2", [64, S]),
        ):
            dmp[nm] = nc.dram_tensor(nm, shp, dt.bfloat16, kind="ExternalOutput")

    with TileContext(nc) as tc:
        with tc.tile_pool(name="persist", bufs=1) as pers:
            # ---- persistent SBUF tiles ----
            XT = pers.tile([128, NK6 * S], dt.bfloat16)
            qT01 = pers.tile([128, S], dt.bfloat16)
            kT01 = pers.tile([128, S], dt.bfloat16)
            qk2T = pers.tile([128, S], dt.bfloat16)   # rows 0-63 q2, 64-127 k2
            Vn = pers.tile([128, NCHUNK * VW], dt.bfloat16)
            Vg_glob = pers.tile([128, VW], dt.bfloat16)
            expTg = pers.tile([128, HPC * S], dt.bfloat16)
            VnZlo = pers.tile([128, NCHUNK * VW], dt.bfloat16)
            VnZhi = pers.tile([128, NCHUNK * VW], dt.bfloat16)
            ctxT01 = pers.tile([128, S], dt.bfloat16)
            ctxT2 = pers.tile([64, S], dt.bfloat16)
            WQTs = pers.tile([128, NK6 * 128], dt.bfloat16)
            WKTs = pers.tile([128, NK6 * 128], dt.bfloat16)
            WQK2s = pers.tile([128, NK6 * 128], dt.bfloat16)
            WVTs = pers.tile([128, NK6 * 192], dt.bfloat16)
            WOT01 = pers.tile([128, HID], dt.bfloat16)
            WOT2 = pers.tile([64, HID], dt.bfloat16)
            BQs = pers.tile([128, 1], dt.float32)
            BKs = pers.tile([128, 1], dt.float32)
            BQK2s = pers.tile([128, 1], dt.float32)
            ONES = pers.tile([1, 64], dt.bfloat16)
            # zero-padded per-head global key/query blocks {0, 63}:
            # kgz{h} = [kg_h; 0] (or [0; kg_1]) so a full-128 contraction
            # against the un-split qT01/qk2T streams selects head h.
            kgz0 = pers.tile([128, 128], dt.bfloat16)
            kgz1 = pers.tile([128, 128], dt.bfloat16)
            kgz2 = pers.tile([128, 128], dt.bfloat16)
            qgz0 = pers.tile([128, 128], dt.bfloat16)
            qgz1 = pers.tile([128, 128], dt.bfloat16)
            qgz2 = pers.tile([128, 128], dt.bfloat16)
            # Zero-padded sparse-score stationaries: kz0 = [k0; 0],
            # kz1 = [0; k1] (dedicated tiles, ready mid-stage-C) and
            # kz2 = [k2; 0] in XT cols 5S..6S (dead after stage C); the
            # moving streams are the raw qT01/qk2T. expTE overlays XT
            # cols 0..S during stage E.
            kz0 = pers.tile([128, S], dt.bfloat16)
            kz1 = pers.tile([128, S], dt.bfloat16)

            # ---- load weights/constants ----
            # Critical-path weights (WQ + bias) go first on the HWDGE
            # queues, then the X chunks alternate sync/scalar so the Q
            # projection can start as soon as chunk 0 lands. Remaining
            # weights ride the otherwise-idle gpsimd (SWDGE) queue.
            nc.sync.dma_start(
                WQTs[:].rearrange("p (c n) -> p c n", c=NK6),
                WQT_d.ap().rearrange("(c p) n -> p c n", p=128),
            )
            nc.scalar.dma_start(BQs[:], BQ_d[:, :])
            # X chunks sliced in exactly the order the Q projection
            # consumes them (sh-half major, c6 minor) so each accumulation
            # pass streams right behind the DMA front
            for sh in range(2):
                for c6 in range(NK6):
                    eng = nc.sync if (c6 % 2 == 0) else nc.scalar
                    eng.dma_start(
                        XT[:, c6 * S + sh * 2048 : c6 * S + (sh + 1) * 2048],
                        XT_d[c6 * 128 : (c6 + 1) * 128,
                             sh * 2048 : (sh + 1) * 2048],
                    )
            for wt_sb, wt_d in (
                (WKTs, WKT_d), (WQK2s, WQK2_d), (WVTs, WVT_d),
            ):
                nc.gpsimd.dma_start(
                    wt_sb[:].rearrange("p (c n) -> p c n", c=NK6),
                    wt_d.ap().rearrange("(c p) n -> p c n", p=128),
                )
            nc.gpsimd.dma_start(WOT01[:], WOT_d[0:128, :])
            nc.gpsimd.dma_start(WOT2[:], WOT_d[128:192, :])
            nc.gpsimd.dma_start(BKs[:], BK_d[:, :])
            nc.gpsimd.dma_start(BQK2s[:], BQK2_d[:, :])
            nc.vector.memset(ONES[:], 1.0)
            # zero halves of the global-block operand tiles
            nc.vector.memset(kgz0[64:128, :], 0.0)
            nc.vector.memset(kgz1[0:64, :], 0.0)
            nc.vector.memset(kgz2[64:128, :], 0.0)
            nc.vector.memset(qgz0[64:128, :], 0.0)
            nc.vector.memset(qgz1[0:64, :], 0.0)
            nc.vector.memset(qgz2[64:128, :], 0.0)
            nc.vector.memset(kz0[64:128, :], 0.0)
            nc.vector.memset(kz1[0:64, :], 0.0)

            # ones columns of Vn: cols c*VW + h*65 + 64
            ones_ap = AP(
                Vn[:].tensor, 64,
                [[NCHUNK * VW, 128], [VW, NCHUNK], [65, 3]],
            )
            nc.vector.memset(ones_ap, 1.0)
            nc.vector.memset(VnZlo[64:128, :], 0.0)
            nc.vector.memset(VnZhi[0:64, :], 0.0)
            # (data halves of VnZlo/VnZhi — including their ones columns —
            # are bulk-copied from Vn after the V projection)

            # helpers -------------------------------------------------
            def qstr(h):
                """moving q stream [128, S] for head h (other head's rows
                are killed by zeros in the kz{h} stationary)"""
                return qT01 if h < 2 else qk2T

            def kzt(h):
                """zero-padded stationary k tile [128, S] for head h"""
                return (kz0, kz1)[h] if h < 2 else XT[:, 5 * S : 6 * S]

            def plan(n):
                """Sparse slots of query block n as one chunk-ALIGNED pair
                (direct Vn chunk / contiguous kT cols) plus 4 singles packed
                into regions by parity (even block -> partitions 0-63, odd ->
                64-127). Returns (u, regions): pair = blocks (2u, 2u+1),
                regions = list of (j_even_or_None, j_odd_or_None)."""
                if n % 2 == 1:
                    u, ws = (n - 1) // 2, n + 1
                else:
                    u, ws = n // 2, n - 1
                singles = [ws, idx[n][5], idx[n][6], idx[n][7]]
                ev = [j for j in singles if j % 2 == 0]
                od = [j for j in singles if j % 2 == 1]
                regions = [
                    (ev[i] if i < len(ev) else None,
                     od[i] if i < len(od) else None)
                    for i in range(max(len(ev), len(od)))
                ]
                return u, regions

            if dump and lvl >= 2:
                nc.vector.memset(ctxT01[:], 0.0)
                nc.vector.memset(ctxT2[:], 0.0)

            # ---- pools ----
            # psSc + the SBUF pools span stages C..G (group-0 scores are
            # emitted mid-stage-C so the tensor queue never bubbles at the
            # V->sparse boundary); psP (6 banks) lives only through C, then
            # psCtx+psG take its banks: 6+2 = 3+2+2 = 8.
            import contextlib
            _pools = contextlib.ExitStack()
            psSc = _pools.enter_context(
                tc.tile_pool(name="psSc", bufs=2, space="PSUM"))
            sprob = _pools.enter_context(tc.tile_pool(name="sprob", bufs=11))
            small = _pools.enter_context(tc.tile_pool(name="small", bufs=2))
            rbsp = _pools.enter_context(tc.tile_pool(name="rbs", bufs=2))
            hscp = _pools.enter_context(tc.tile_pool(name="hsc", bufs=2))
            osb = _pools.enter_context(tc.tile_pool(name="osb", bufs=2))

            groups = [(1 + 8 * g, min(8 * g + 8, 62)) for g in range(8)]
            plans = {n: plan(n) for n in range(1, 63)} if lvl >= 2 else {}

            def emit_scores(n0, n1, h):
                """Scores + exp for query blocks n0..n1, head h. Batches
                into [128,512] psum tiles; per block: aligned pair at its
                base col, then parity-packed single regions. Returns
                (pair_ref, single_ref) mapping into the exp'd SBUF tiles."""
                qt = qstr(h)
                kt = kzt(h)
                pair_ref = {}    # n -> (tile, col)
                single_ref = {}  # (n, i) -> (tile, col)
                batch = []       # [(kind, key, col)]
                cur = 0
                sc = None

                def flush():
                    nonlocal batch, cur, sc
                    if sc is None:
                        return
                    expt = sprob.tile([128, 512], dt.bfloat16, tag="pr")
                    nc.scalar.activation(
                        expt[:, 0:cur], sc[:, 0:cur],
                        mybir.ActivationFunctionType.Exp,
                    )
                    for kind, key, c_ in batch:
                        if kind == 0:
                            pair_ref[key] = (expt, c_)
                        else:
                            single_ref[key] = (expt, c_)
                    batch, cur, sc = [], 0, None

                def alloc(width):
                    nonlocal cur, sc
                    if cur + width > 512:
                        flush()
                    if sc is None:
                        sc = psSc.tile([128, 512], dt.float32, tag="sc")
                    c_ = cur
                    cur += width
                    return c_

                for un in build_units(n0, n1):
                    pcol = alloc(64 * len(un))
                    nc.tensor.matmul(
                        sc[:, pcol : pcol + 64 * len(un)],
                        kt[:, 2 * plans[un[0]][0] * 64
                           : 2 * plans[un[0]][0] * 64 + 128],
                        qt[:, un[0] * 64 : (un[0] + len(un)) * 64],
                    )
                    for ui, n_ in enumerate(un):
                        batch.append((0, n_, pcol + 64 * ui))
                    for n_ in un:
                        rhs = qt[:, n_ * 64 : (n_ + 1) * 64]
                        for si, (je, jo) in enumerate(plans[n_][1]):
                            rcol = alloc(64)
                            if je is not None and jo is not None:
                                nc.tensor.matmul(
                                    sc[0:64, rcol : rcol + 64],
                                    kt[:, je * 64 : je * 64 + 64],
                                    rhs,
                                )
                                nc.tensor.matmul(
                                    sc[64:128, rcol : rcol + 64],
                                    kt[:, jo * 64 : jo * 64 + 64],
                                    rhs,
                                )
                            else:
                                # one-sided: fill BOTH halves with a
                                # 128-wide kT slice so the dead half
                                # holds bounded scores
                                ja = je if je is not None else jo - 1
                                nc.tensor.matmul(
                                    sc[:, rcol : rcol + 64],
                                    kt[:, ja * 64 : ja * 64 + 128],
                                    rhs,
                                )
                            batch.append((1, (n_, si), rcol))
                flush()
                return pair_ref, single_ref

            def build_units(n0, n1):
                """(2u, 2u+1) block pairs share the same aligned key pair
                -> merged score/ctx matmuls"""
                units = []
                n = n0
                while n <= n1:
                    if n % 2 == 0 and n + 1 <= n1:
                        units.append((n, n + 1))
                        n += 2
                    else:
                        units.append((n,))
                        n += 1
                return units

            pre_refs = {}

            # ---- stage C: projections ----
            with tc.tile_pool(name="psP", bufs=6, space="PSUM") as psP:
                def proj128(wt_sb, dst, bias):
                    for sh in range(2):
                        pts = [
                            psP.tile([128, 512], dt.float32, tag="pp",
                                     name=f"pp{i}")
                            for i in range(4)
                        ]
                        for c6 in range(NK6):
                            lhs = wt_sb[:, c6 * 128 : (c6 + 1) * 128]
                            for nb in range(4):
                                nc.tensor.matmul(
                                    pts[nb][:],
                                    lhs,
                                    XT[:, c6 * S + sh * 2048 + nb * 512
                                       : c6 * S + sh * 2048 + (nb + 1) * 512],
                                    start=(c6 == 0),
                                    stop=(c6 == NK6 - 1),
                                )
                        for nb in range(4):
                            col = sh * 2048 + nb * 512
                            nc.scalar.activation(
                                dst[:, col : col + 512],
                                pts[nb][:],
                                mybir.ActivationFunctionType.Identity,
                                bias=bias[:],
                                scale=1.0,
                            )

                proj128(WQTs, qT01, BQs)
                proj128(WKTs, kT01, BKs)
                # kz0/kz1 data halves ride the idle gpsimd queue so the
                # tiny kgz/qgz copies below (which gate dglob) stay at the
                # head of the sync queue
                nc.gpsimd.dma_start(kz0[0:64, :], kT01[0:64, :])
                nc.gpsimd.dma_start(kz1[64:128, :], kT01[64:128, :])
                for sl, j in enumerate((0, NB - 1)):
                    nc.sync.dma_start(
                        kgz0[0:64, sl * 64 : sl * 64 + 64],
                        kT01[0:64, j * 64 : (j + 1) * 64],
                    )
                    nc.sync.dma_start(
                        kgz1[64:128, sl * 64 : sl * 64 + 64],
                        kT01[64:128, j * 64 : (j + 1) * 64],
                    )
                    nc.sync.dma_start(
                        qgz0[0:64, sl * 64 : sl * 64 + 64],
                        qT01[0:64, j * 64 : (j + 1) * 64],
                    )
                    nc.sync.dma_start(
                        qgz1[64:128, sl * 64 : sl * 64 + 64],
                        qT01[64:128, j * 64 : (j + 1) * 64],
                    )

                # ---- global key scores (blocks 0,63 vs all queries) ----
                def dglob(h):
                    lhs = (kgz0, kgz1, kgz2)[h]
                    # raw q streams (the duplicated qd views in XT are not
                    # filled until after the V projection)
                    qt = qT01 if h < 2 else qk2T
                    for kb in range(8):
                        ps = psP.tile([128, 512], dt.float32, tag="pp")
                        nc.tensor.matmul(
                            ps[:], lhs[:], qt[:, kb * 512 : (kb + 1) * 512],
                        )
                        nc.scalar.activation(
                            expTg[:, h * S + kb * 512 : h * S + (kb + 1) * 512],
                            ps[:],
                            mybir.ActivationFunctionType.Exp,
                        )

                if lvl >= 2:
                    dglob(0)
                    dglob(1)

                proj128(WQK2s, qk2T, BQK2s)
                for sl, j in enumerate((0, NB - 1)):
                    nc.sync.dma_start(
                        kgz2[0:64, sl * 64 : sl * 64 + 64],
                        qk2T[64:128, j * 64 : (j + 1) * 64],
                    )
                    nc.sync.dma_start(
                        qgz2[0:64, sl * 64 : sl * 64 + 64],
                        qk2T[0:64, j * 64 : (j + 1) * 64],
                    )

                if lvl >= 2:
                    dglob(2)
                    # group-0 h0/h1 scores now: their matmuls precede the
                    # V projection in the tensor FIFO, their exps overlap
                    # V's matmuls, and ctx can start the moment V drains
                    pre_refs[(0, 0)] = emit_scores(*groups[0], 0)
                    pre_refs[(0, 1)] = emit_scores(*groups[0], 1)

                # V natural with interleaved ones columns
                for qt4 in range(8):
                    pvs = [
                        psP.tile([128, 192], dt.float32, tag="pp",
                                 name=f"pv{i}")
                        for i in range(4)
                    ]
                    for c6 in range(NK6):
                        for i in range(4):
                            t = qt4 * 4 + i
                            nc.tensor.matmul(
                                pvs[i][:],
                                XT[:, c6 * S + t * 128 : c6 * S + (t + 1) * 128],
                                WVTs[:, c6 * 192 : (c6 + 1) * 192],
                                start=(c6 == 0),
                                stop=(c6 == NK6 - 1),
                            )
                    for i in range(4):
                        t = qt4 * 4 + i
                        base = Vn[0:128, t * VW : t * VW + 64]
                        dst = AP(base.tensor, base.offset,
                                 [[NCHUNK * VW, 128], [65, 3], [1, 64]])
                        nc.vector.tensor_copy(
                            dst,
                            pvs[i][:].rearrange("p (a b) -> p a b", a=3),
                        )
                    # incremental masked-V copies (with ones columns): ride
                    # behind each qt4 chunk so VnZ is ready ~when V drains
                    c0, c1 = qt4 * 4 * VW, (qt4 + 1) * 4 * VW
                    nc.sync.dma_start(
                        VnZlo[0:64, c0:c1], Vn[0:64, c0:c1]
                    )
                    nc.scalar.dma_start(
                        VnZhi[64:128, c0:c1], Vn[64:128, c0:c1]
                    )

            # ---- post-V fills (XT tail is WAR-free from here) ----
            # Vg_glob: V rows of blocks 0 and 63 (with ones) — gates the
            # first ctx matmul of every group
            nc.sync.dma_start(Vg_glob[0:64, :], Vn[0:64, 0:VW])
            nc.scalar.dma_start(
                Vg_glob[64:128, :], Vn[64:128, 31 * VW : 32 * VW]
            )
            # kz2 = [k2; 0] (h2 runs third in each group — off-path)
            nc.gpsimd.dma_start(XT[0:64, 5 * S : 6 * S], qk2T[64:128, :])
            nc.vector.memset(XT[64:128, 5 * S : 6 * S], 0.0)

            # ---- stages D (sparse), E (global rows), G (out proj) ----
            with (
                tc.tile_pool(name="psCtx", bufs=3, space="PSUM") as psCtx,
                tc.tile_pool(name="psG", bufs=2, space="PSUM") as psG,
            ):
                def emit_norm(ctxp, h, c0, w, gap=False):
                    """Normalize ctxp [65, w] -> ctxT head h cols c0..c0+w.
                    gap=True: dst cols are blocks {0, 63} (w=128)."""
                    drow = small.tile([1, 512], dt.float32, tag="dr")
                    nc.scalar.activation(
                        drow[:, 0:w], ctxp[64:65, 0:w],
                        mybir.ActivationFunctionType.Identity,
                    )
                    rr32 = small.tile([1, 512], dt.float32, tag="rr32")
                    nc.vector.reciprocal_approx_fast(
                        rr32[:, 0:w], drow[:, 0:w]
                    )
                    rrbf = small.tile([1, 512], dt.bfloat16, tag="rr")
                    nc.vector.tensor_copy(rrbf[:, 0:w], rr32[:, 0:w])
                    rbc = psSc.tile([64, 512], dt.float32, tag="sc", name="rbc")
                    nc.tensor.matmul(
                        rbc[:, 0:w], ONES[:], rrbf[:, 0:w],
                    )
                    rbs = rbsp.tile([64, 512], dt.float32, tag="rbs")
                    nc.vector.tensor_copy(rbs[:, 0:w], rbc[:, 0:w])
                    if h == 2:
                        dstt, p0 = ctxT2, 0
                    else:
                        dstt, p0 = ctxT01, 64 * h
                    if gap:
                        if h == 1:
                            hs = hscp.tile([64, 512], dt.bfloat16, tag="hs")
                            nc.vector.tensor_mul(
                                hs[:, 0:w], ctxp[0:64, 0:w], rbs[:, 0:w]
                            )
                            base = dstt[64:128, 0:64]
                            dap = AP(base.tensor, base.offset,
                                     [[S, 64], [(NB - 1) * 64, 2], [1, 64]])
                            nc.sync.dma_start(dap, hs[:, 0:w])
                        else:
                            base = dstt[p0 : p0 + 64, 0:64]
                            dap = AP(base.tensor, base.offset,
                                     [[S, 64], [(NB - 1) * 64, 2], [1, 64]])
                            nc.vector.tensor_mul(
                                dap, ctxp[0:64, 0:w], rbs[:, 0:w]
                            )
                    else:
                        if h == 1:
                            hs = hscp.tile([64, 512], dt.bfloat16, tag="hs")
                            nc.vector.tensor_mul(
                                hs[:, 0:w], ctxp[0:64, 0:w], rbs[:, 0:w]
                            )
                            nc.sync.dma_start(
                                dstt[64:128, c0 : c0 + w], hs[:, 0:w]
                            )
                        else:
                            nc.vector.tensor_mul(
                                dstt[p0 : p0 + 64, c0 : c0 + w],
                                ctxp[0:64, 0:w], rbs[:, 0:w],
                            )

                def emit_g_chunk(t):
                    """Output projection for s-chunk t (128 rows)."""
                    pos = [
                        psG.tile([128, 384], dt.float32, tag="po",
                                 name=f"po{i}")
                        for i in range(2)
                    ]
                    for nh in range(2):
                        nc.tensor.matmul(
                            pos[nh][:],
                            ctxT01[:, t * 128 : (t + 1) * 128],
                            WOT01[:, nh * 384 : (nh + 1) * 384],
                            start=True, stop=False,
                        )
                        nc.tensor.matmul(
                            pos[nh][:],
                            ctxT2[:, t * 128 : (t + 1) * 128],
                            WOT2[:, nh * 384 : (nh + 1) * 384],
                            start=False, stop=True,
                        )
                    ot = osb.tile([128, HID], dt.float16, tag="ot")
                    nc.vector.tensor_copy(ot[:, 0:384], pos[0][:])
                    nc.vector.tensor_copy(ot[:, 384:768], pos[1][:])
                    nc.sync.dma_start(OUT_d[t * 128 : (t + 1) * 128, :], ot[:])

                norm_q = []           # pending (ctxp, h, c0, w, gap), depth 2
                next_g = 1            # next output chunk to emit (skip 0, 31)

                def emit_E():
                    # stage E: global query rows (blocks 0, 63), dense.
                    # expTE overlays XT cols 0..S per head is too big —
                    # use thirds of XT[0:3S] so all heads coexist; the
                    # three ctx chains then interleave (c outer, h inner)
                    # for better exp/matmul pipelining at the tail.
                    while norm_q:
                        emit_norm(*norm_q.pop(0))
                    for h in range(HPC):
                        kt = kzt(h)
                        qg = (qgz0, qgz1, qgz2)[h]
                        expTE = XT[:, h * S : (h + 1) * S]
                        for kb in range(8):
                            sc = psSc.tile([128, 512], dt.float32, tag="sc")
                            for i in range(4):
                                c = kb * 4 + i
                                nc.tensor.matmul(
                                    sc[:, i * 128 : (i + 1) * 128],
                                    kt[:, c * 128 : (c + 1) * 128],
                                    qg[:],
                                )
                            nc.scalar.activation(
                                expTE[:, kb * 512 : (kb + 1) * 512], sc[:],
                                mybir.ActivationFunctionType.Exp,
                            )
                    ctxes = [
                        psCtx.tile([65, 512], dt.float32, tag="cx",
                                   name=f"cxe{h}")
                        for h in range(HPC)
                    ]
                    for c in range(NCHUNK):
                        for h in range(HPC):
                            nc.tensor.matmul(
                                ctxes[h][:, 0:128],
                                Vn[:, c * VW + h * 65 : c * VW + h * 65 + 65],
                                XT[:, h * S + c * 128 : h * S + (c + 1) * 128],
                                start=(c == 0), stop=(c == NCHUNK - 1),
                            )
                    for h in range(HPC):
                        norm_q.append((ctxes[h], h, 0, 128, True))

                if lvl >= 2:
                    for gi, (n0, n1) in enumerate(groups):
                        for h in range(HPC):
                            if (gi, h) in pre_refs:
                                pair_ref, single_ref = pre_refs.pop((gi, h))
                            else:
                                pair_ref, single_ref = emit_scores(n0, n1, h)
                            units = build_units(n0, n1)

                            # lagged normalize (2 steps behind)
                            while len(norm_q) >= 2:
                                emit_norm(*norm_q.pop(0))

                            # ctx accumulation, glob-first: ONE wide N=w
                            # start matmul seeds every region's has_written
                            # bits (single start per bank — safe), singles
                            # accumulate, and each unit's merged pair matmul
                            # closes its regions with stop=True.
                            w = (n1 - n0 + 1) * 64
                            ctxp = psCtx.tile([65, 512], dt.float32, tag="cx")
                            nc.tensor.matmul(
                                ctxp[:, 0:w],
                                Vg_glob[:, h * 65 : h * 65 + 65],
                                expTg[:, h * S + n0 * 64
                                      : h * S + n0 * 64 + w],
                                start=True, stop=False,
                                skip_group_check=True,
                            )
                            for un in units:
                                for n_ in un:
                                    col = (n_ - n0) * 64
                                    for i, (je, jo) in enumerate(plans[n_][1]):
                                        set_, ecol = single_ref[(n_, i)]
                                        if je is not None:
                                            nc.tensor.matmul(
                                                ctxp[:, col : col + 64],
                                                VnZlo[:, (je // 2) * VW + h * 65
                                                      : (je // 2) * VW + h * 65 + 65],
                                                set_[:, ecol : ecol + 64],
                                                start=False, stop=False,
                                                skip_group_check=True,
                                            )
                                        if jo is not None:
                                            nc.tensor.matmul(
                                                ctxp[:, col : col + 64],
                                                VnZhi[:, (jo // 2) * VW + h * 65
                                                      : (jo // 2) * VW + h * 65 + 65],
                                                set_[:, ecol : ecol + 64],
                                                start=False, stop=False,
                                                skip_group_check=True,
                                            )
                                # closing pair matmul over the whole unit
                                # (scores allocated the unit as one
                                # contiguous region, so expt/pc are shared)
                                u = plans[un[0]][0]
                                expt, pc = pair_ref[un[0]]
                                col0 = (un[0] - n0) * 64
                                nc.tensor.matmul(
                                    ctxp[:, col0 : col0 + 64 * len(un)],
                                    Vn[:, u * VW + h * 65
                                       : u * VW + h * 65 + 65],
                                    expt[:, pc : pc + 64 * len(un)],
                                    start=False, stop=True,
                                    skip_group_check=True,
                                )
                            norm_q.append((ctxp, h, n0 * 64, w, False))

                            # sprinkle dense output-projection chunks (N=384,
                            # full-density matmuls) between heads so every
                            # HAM activity window sees high-duty work — keeps
                            # the PE clock at K=8/8 through the sparse stage.
                            # norm_q lags 2 behind: at h=0 the previous
                            # group's h2 norm is still pending, so only
                            # gi-2 is fully normalized then.
                            done_gi = gi - 2 if h == 0 else gi - 1
                            if done_gi >= 0:
                                done_n = groups[done_gi][1]
                                emitted = 0
                                while (next_g < 31 and emitted < 2
                                       and 2 * next_g + 1 <= done_n):
                                    emit_g_chunk(next_g)
                                    next_g += 1
                                    emitted += 1

                if lvl >= 3:
                    emit_E()

                while norm_q:
                    emit_norm(*norm_q.pop(0))

                # ---- stage G tail ----
                if lvl >= 4:
                    while next_g < 31:
                        emit_g_chunk(next_g)
                        next_g += 1
                    emit_g_chunk(0)
                    emit_g_chunk(31)
                else:
                    zt = osb.tile([128, HID], dt.float16, tag="ot")
                    nc.vector.memset(zt[:], 0.0)
                    for t in range(NCHUNK):
                        nc.scalar.dma_start(
                            OUT_d[t * 128 : (t + 1) * 128, :], zt[:]
                        )

                if dump:
                    nc.sync.dma_start(dmp["DQT01"].ap(), qT01[:])
                    nc.sync.dma_start(dmp["DKT01"].ap(), kT01[:])
                    nc.sync.dma_start(dmp["DQK2"].ap(), qk2T[:])
                    nc.sync.dma_start(
                        dmp["DK2LO"].ap(), XT[0:64, 5 * S : 6 * S]
                    )
                    nc.sync.dma_start(dmp["DVN"].ap(), Vn[:])
                    if lvl >= 2:
                        nc.sync.dma_start(dmp["DCTXT01"].ap(), ctxT01[:])
                        nc.sync.dma_start(dmp["DCTXT2"].ap(), ctxT2[:])

            _pools.close()

    nc.finalize()
    return nc


def kernel(X, band_mask, from_mask, to_mask, blocked_encoder_mask, rand_idx,
           Wq_w, Wq_b, Wk_w, Wk_b, Wv_w, Wv_b, Wo_w, Wo_b):
    from concourse.bass_utils import run_bass_kernel_spmd

    X = np.asarray(X, dtype=np.float32)
    rand_idx = np.asarray(rand_idx)
    Wq_w = np.asarray(Wq_w, np.float32); Wq_b = np.asarray(Wq_b, np.float32)
    Wk_w = np.asarray(Wk_w, np.float32); Wk_b = np.asarray(Wk_b, np.float32)
    Wv_w = np.asarray(Wv_w, np.float32); Wv_b = np.asarray(Wv_b, np.float32)
    Wo_w = np.asarray(Wo_w, np.float32); Wo_b = np.asarray(Wo_b, np.float32)

    blk = np.arange(NB)
    window = (blk[:, None] + np.array([-1, 0, 1])[None, :]) % NB
    glob = np.broadcast_to(np.array([0, NB - 1]), (NB, 2))
    idx = np.concatenate([window, glob, rand_idx.astype(np.int64)], axis=1)
    upto = os.environ.get("KUPTO", "G")
    key = (idx.tobytes(), upto, os.environ.get("KDUMP", "0"))
    if key not in _cache:
        _cache[key] = _build(
            tuple(tuple(int(v) for v in row) for row in idx), upto=upto
        )
    nc = _cache[key]

    sc = 1.0 / np.sqrt(HD)
    in_maps = []
    for c in range(NCORES):
        b = c // 4
        h0 = HPC * (c % 4)
        s01 = slice(h0 * HD, (h0 + 2) * HD)
        s2 = slice((h0 + 2) * HD, (h0 + 3) * HD)
        hsl = slice(h0 * HD, (h0 + HPC) * HD)
        wqk2 = np.concatenate(
            [(Wq_w[s2, :] * sc).T, Wk_w[s2, :].T], axis=1
        )  # [768, 128]
        bqk2 = np.concatenate([Wq_b[s2] * sc, Wk_b[s2]])
        in_maps.append({
            "XT": np.ascontiguousarray(X[b].T).astype(BF16),
            "WQT": np.ascontiguousarray((Wq_w[s01, :] * sc).T).astype(BF16),
            "WKT": np.ascontiguousarray(Wk_w[s01, :].T).astype(BF16),
            "WQK2": np.ascontiguousarray(wqk2).astype(BF16),
            "WVT": np.ascontiguousarray(Wv_w[hsl, :].T).astype(BF16),
            "WOT": np.ascontiguousarray(Wo_w[:, hsl].T).astype(BF16),
            "BQ": (Wq_b[s01] * sc).astype(np.float32)[:, None],
            "BK": Wk_b[s01].astype(np.float32)[:, None],
            "BQK2": bqk2.astype(np.float32)[:, None],
        })

    global _last_in_maps
    _last_in_maps = in_maps
    res = run_bass_kernel_spmd(nc, in_maps, core_ids=list(range(NCORES)))

    out = np.zeros((B, S, HID), dtype=np.float32)
    for c in range(NCORES):
        out[c // 4] += res.results[c]["OUT"].astype(np.float32)
    global _last_res
    _last_res = res
    # bias terms handled on host: Wo bias, and Wv bias pushed through Wo
    # (sum_k probs = 1, so ctx picks up Wv_b exactly).
    out += (Wo_w @ Wv_b + Wo_b)[None, None, :]
    fm = np.asarray(from_mask, np.float32).reshape(B, S)
    if not np.all(fm == 1.0):
        raise NotImplementedError("kernel assumes all-ones from_mask")
    return out



# revision 68
# speedup vs baseline: 1.0378x; 1.0378x over previous
"""BigBird block-sparse attention on 8 Trainium2 NeuronCores (v3).

Sharding: core c handles batch b = c // 4 and 3 heads starting at 3 * (c % 4).
Each core computes its partial output projection OUT_c = sum_h ctx_h @ WoT_h
(fp16, [4096, 768]); the host sums the 4 partials per batch and adds the
bias terms (Wo_b plus the Wv bias pushed through the output projection).

Design: scores are computed directly in transposed layout (scoresT
[keys, queries]) so no PE transposes of probabilities are needed. Key
blocks are processed in PAIRS stacked on 128 PSUM partitions. V is kept
in natural layout with an interleaved ones column per head
([h0|1|h1|1|h2|1] = 195 cols/chunk); the ctx matmul lhs [128 keys, 65]
then accumulates the softmax denominator as output row 64 for free.
Normalization per group of 8 query blocks: reciprocal of the denominator
row, broadcast via a C=1 PE matmul, one DVE multiply into ctxT.

v3 performance structure (from perfetto/HAM analysis):
- X is transposed on the HOST and loaded via plain sliced DMA in matmul
  consumption order (dma_start_transpose serializes against all other
  DMA traffic and cost ~25us of startup).
- All sparse/global score matmuls use FULL 128-partition contraction via
  zero-padded stationaries kz{h} = [k_h; 0] / [0; k_1] (zeros select the
  head out of the shared qT01/qk2T streams). Half-contraction matmuls
  left the PE activity monitor (HAM) seeing an idle array: it held the
  clock gate at K=4/8 (1.2 GHz) through the whole sparse stage — the
  single biggest win (~70us).
- Glob-first ctx: one wide N=512 start matmul per group seeds PSUM
  has_written, singles accumulate, the merged pair matmul closes.
- Group-0 h0/h1 scores are emitted mid-stage-C (before the V projection)
  so the tensor FIFO never bubbles at the C->D boundary; PSUM pools are
  phased (psP 6 banks in C; psCtx+psG after) to make room for psSc+sprob
  across the boundary.
- Dense N=384 output-projection chunks are sprinkled between heads
  inside the sparse loop to keep HAM activity windows above threshold.
- Masked-V copies (VnZlo/VnZhi) ride incrementally behind the V
  projection as bulk SBUF->SBUF DMAs instead of 64 strided DVE copies.
Remaining run-to-run variance (~260-310us) tracks HAM/thermal clock
state, not schedule changes.
"""

import os

import numpy as np
import ml_dtypes

B, S, HID = 2, 4096, 768
NH, HD = 12, 64
BS = 64
NB = S // BS            # 64 blocks
NR = 3
NCORES = 8
HPC = 3                 # heads per core
NCHUNK = S // 128       # 32 s-chunks of 128
NK6 = HID // 128        # 6 hid-chunks
VW = 3 * 65             # Vn cols per chunk: [h0(64)|1][h1(64)|1][h2(64)|1]

BF16 = ml_dtypes.bfloat16

_cache = {}
_last_in_maps = None


def _pair4(vals):
    """Pair up 4 key-block ids, avoiding equal pairs when possible.
    Returns two (ja, jb) pairs with ja <= jb."""
    s = sorted(vals)
    if s[0] == s[1] and s[1] != s[2]:
        return [(s[0], s[2]), (s[1], s[3])]
    if s[2] == s[3] and s[1] != s[2]:
        return [(s[0], s[2]), (s[1], s[3])]
    return [(s[0], s[1]), (s[2], s[3])]


def _build(idx, upto="G"):
    """Build the SPMD Bass program. idx: tuple of NB tuples of K key-block
    ids (window 3, global 2, rand 3 — only rand entries [5:8] are used;
    window/global are recomputed here).

    upto: last stage to include ("C", "DG", "D", "E", "G") for bisection.
    """
    import concourse.mybir as mybir
    from concourse import bacc
    from concourse.tile import TileContext
    from bass_rust import AP

    dt = mybir.dt
    order = {"C": 0, "DG": 1, "D": 2, "E": 3, "G": 4}
    lvl = order[upto]
    dump = os.environ.get("KDUMP", "0") == "1"
    nc = bacc.Bacc()

    # ---- DRAM tensors ----
    # XT is X pre-transposed on the host: [HID, S]. Plain row-major DMA
    # (8KB/partition lines) replaces the serialized dma_start_transpose.
    XT_d = nc.dram_tensor("XT", [HID, S], dt.bfloat16, kind="ExternalInput")
    WQT_d = nc.dram_tensor("WQT", [HID, 128], dt.bfloat16, kind="ExternalInput")
    WKT_d = nc.dram_tensor("WKT", [HID, 128], dt.bfloat16, kind="ExternalInput")
    WQK2_d = nc.dram_tensor("WQK2", [HID, 128], dt.bfloat16, kind="ExternalInput")
    WVT_d = nc.dram_tensor("WVT", [HID, 192], dt.bfloat16, kind="ExternalInput")
    WOT_d = nc.dram_tensor("WOT", [192, HID], dt.bfloat16, kind="ExternalInput")
    BQ_d = nc.dram_tensor("BQ", [128, 1], dt.float32, kind="ExternalInput")
    BK_d = nc.dram_tensor("BK", [128, 1], dt.float32, kind="ExternalInput")
    BQK2_d = nc.dram_tensor("BQK2", [128, 1], dt.float32, kind="ExternalInput")
    OUT_d = nc.dram_tensor("OUT", [S, HID], dt.float16, kind="ExternalOutput")
    dmp = {}
    if dump:
        for nm, shp in (
            ("DQT01", [128, S]), ("DKT01", [128, S]), ("DQK2", [128, S]),
            ("DK2LO", [64, S]), ("DVN", [128, NCHUNK * VW]),
            ("DCTXT01", [128, S]), ("DCTXT2", [64, S]),
        ):
            dmp[nm] = nc.dram_tensor(nm, shp, dt.bfloat16, kind="ExternalOutput")

    with TileContext(nc) as tc:
        with tc.tile_pool(name="persist", bufs=1) as pers:
            # ---- persistent SBUF tiles ----
            XT = pers.tile([128, NK6 * S], dt.bfloat16)
            qT01 = pers.tile([128, S], dt.bfloat16)
            kT01 = pers.tile([128, S], dt.bfloat16)
            qk2T = pers.tile([128, S], dt.bfloat16)   # rows 0-63 q2, 64-127 k2
            Vn = pers.tile([128, NCHUNK * VW], dt.bfloat16)
            Vg_glob = pers.tile([128, VW], dt.bfloat16)
            expTg = pers.tile([128, HPC * S], dt.bfloat16)
            VnZlo = pers.tile([128, NCHUNK * VW], dt.bfloat16)
            VnZhi = pers.tile([128, NCHUNK * VW], dt.bfloat16)
            ctxT01 = pers.tile([128, S], dt.bfloat16)
            ctxT2 = pers.tile([64, S], dt.bfloat16)
            WQTs = pers.tile([128, NK6 * 128], dt.bfloat16)
            WKTs = pers.tile([128, NK6 * 128], dt.bfloat16)
            WQK2s = pers.tile([128, NK6 * 128], dt.bfloat16)
            WVTs = pers.tile([128, NK6 * 192], dt.bfloat16)
            WOT01 = pers.tile([128, HID], dt.bfloat16)
            WOT2 = pers.tile([64, HID], dt.bfloat16)
            BQs = pers.tile([128, 1], dt.float32)
            BKs = pers.tile([128, 1], dt.float32)
            BQK2s = pers.tile([128, 1], dt.float32)
            ONES = pers.tile([1, 64], dt.bfloat16)
            # zero-padded per-head global key/query blocks {0, 63}:
            # kgz{h} = [kg_h; 0] (or [0; kg_1]) so a full-128 contraction
            # against the un-split qT01/qk2T streams selects head h.
            kgz0 = pers.tile([128, 128], dt.bfloat16)
            kgz1 = pers.tile([128, 128], dt.bfloat16)
            kgz2 = pers.tile([128, 128], dt.bfloat16)
            qgz0 = pers.tile([128, 128], dt.bfloat16)
            qgz1 = pers.tile([128, 128], dt.bfloat16)
            qgz2 = pers.tile([128, 128], dt.bfloat16)
            # Zero-padded sparse-score stationaries: kz0 = [k0; 0],
            # kz1 = [0; k1] (dedicated tiles, ready mid-stage-C) and
            # kz2 = [k2; 0] in XT cols 5S..6S (dead after stage C); the
            # moving streams are the raw qT01/qk2T. expTE overlays XT
            # cols 0..S during stage E.
            kz0 = pers.tile([128, S], dt.bfloat16)
            kz1 = pers.tile([128, S], dt.bfloat16)

            # ---- load weights/constants ----
            # Critical-path weights (WQ + bias) go first on the HWDGE
            # queues, then the X chunks alternate sync/scalar so the Q
            # projection can start as soon as chunk 0 lands. Remaining
            # weights ride the otherwise-idle gpsimd (SWDGE) queue.
            nc.sync.dma_start(
                WQTs[:].rearrange("p (c n) -> p c n", c=NK6),
                WQT_d.ap().rearrange("(c p) n -> p c n", p=128),
            )
            nc.scalar.dma_start(BQs[:], BQ_d[:, :])
            # X chunks sliced in exactly the order the Q projection
            # consumes them (sh-half major, c6 minor) so each accumulation
            # pass streams right behind the DMA front
            for sh in range(2):
                for c6 in range(NK6):
                    eng = nc.sync if (c6 % 2 == 0) else nc.scalar
                    eng.dma_start(
                        XT[:, c6 * S + sh * 2048 : c6 * S + (sh + 1) * 2048],
                        XT_d[c6 * 128 : (c6 + 1) * 128,
                             sh * 2048 : (sh + 1) * 2048],
                    )
            for wt_sb, wt_d in (
                (WKTs, WKT_d), (WQK2s, WQK2_d), (WVTs, WVT_d),
            ):
                nc.gpsimd.dma_start(
                    wt_sb[:].rearrange("p (c n) -> p c n", c=NK6),
                    wt_d.ap().rearrange("(c p) n -> p c n", p=128),
                )
            nc.gpsimd.dma_start(WOT01[:], WOT_d[0:128, :])
            nc.gpsimd.dma_start(WOT2[:], WOT_d[128:192, :])
            nc.gpsimd.dma_start(BKs[:], BK_d[:, :])
            nc.gpsimd.dma_start(BQK2s[:], BQK2_d[:, :])
            nc.vector.memset(ONES[:], 1.0)
            # zero halves of the global-block operand tiles
            nc.vector.memset(kgz0[64:128, :], 0.0)
            nc.vector.memset(kgz1[0:64, :], 0.0)
            nc.vector.memset(kgz2[64:128, :], 0.0)
            nc.vector.memset(qgz0[64:128, :], 0.0)
            nc.vector.memset(qgz1[0:64, :], 0.0)
            nc.vector.memset(qgz2[64:128, :], 0.0)
            nc.vector.memset(kz0[64:128, :], 0.0)
            nc.vector.memset(kz1[0:64, :], 0.0)

            # ones columns of Vn: cols c*VW + h*65 + 64
            ones_ap = AP(
                Vn[:].tensor, 64,
                [[NCHUNK * VW, 128], [VW, NCHUNK], [65, 3]],
            )
            nc.vector.memset(ones_ap, 1.0)
            nc.vector.memset(VnZlo[64:128, :], 0.0)
            nc.vector.memset(VnZhi[0:64, :], 0.0)
            # (data halves of VnZlo/VnZhi — including their ones columns —
            # are bulk-copied from Vn after the V projection)

            # helpers -------------------------------------------------
            def qstr(h):
                """moving q stream [128, S] for head h (other head's rows
                are killed by zeros in the kz{h} stationary)"""
                return qT01 if h < 2 else qk2T

            def kzt(h):
                """zero-padded stationary k tile [128, S] for head h"""
                return (kz0, kz1)[h] if h < 2 else XT[:, 5 * S : 6 * S]

            def plan(n):
                """Sparse slots of query block n as one chunk-ALIGNED pair
                (direct Vn chunk / contiguous kT cols) plus 4 singles packed
                into regions by parity (even block -> partitions 0-63, odd ->
                64-127). Returns (u, regions): pair = blocks (2u, 2u+1),
                regions = list of (j_even_or_None, j_odd_or_None)."""
                if n % 2 == 1:
                    u, ws = (n - 1) // 2, n + 1
                else:
                    u, ws = n // 2, n - 1
                singles = [ws, idx[n][5], idx[n][6], idx[n][7]]
                ev = [j for j in singles if j % 2 == 0]
                od = [j for j in singles if j % 2 == 1]
                regions = [
                    (ev[i] if i < len(ev) else None,
                     od[i] if i < len(od) else None)
                    for i in range(max(len(ev), len(od)))
                ]
                return u, regions

            if dump and lvl >= 2:
                nc.vector.memset(ctxT01[:], 0.0)
                nc.vector.memset(ctxT2[:], 0.0)

            # ---- pools ----
            # psSc + the SBUF pools span stages C..G (group-0 scores are
            # emitted mid-stage-C so the tensor queue never bubbles at the
            # V->sparse boundary); psP (6 banks) lives only through C, then
            # psCtx+psG take its banks: 6+2 = 3+2+2 = 8.
            import contextlib
            _pools = contextlib.ExitStack()
            psSc = _pools.enter_context(
                tc.tile_pool(name="psSc", bufs=2, space="PSUM"))
            sprob = _pools.enter_context(tc.tile_pool(name="sprob", bufs=11))
            small = _pools.enter_context(tc.tile_pool(name="small", bufs=2))
            rbsp = _pools.enter_context(tc.tile_pool(name="rbs", bufs=2))
            hscp = _pools.enter_context(tc.tile_pool(name="hsc", bufs=2))
            osb = _pools.enter_context(tc.tile_pool(name="osb", bufs=2))

            groups = [(1 + 8 * g, min(8 * g + 8, 62)) for g in range(8)]
            plans = {n: plan(n) for n in range(1, 63)} if lvl >= 2 else {}

            def emit_scores(n0, n1, h):
                """Scores + exp for query blocks n0..n1, head h. Batches
                into [128,512] psum tiles; per block: aligned pair at its
                base col, then parity-packed single regions. Returns
                (pair_ref, single_ref) mapping into the exp'd SBUF tiles."""
                qt = qstr(h)
                kt = kzt(h)
                pair_ref = {}    # n -> (tile, col)
                single_ref = {}  # (n, i) -> (tile, col)
                batch = []       # [(kind, key, col)]
                cur = 0
                sc = None

                def flush():
                    nonlocal batch, cur, sc
                    if sc is None:
                        return
                    expt = sprob.tile([128, 512], dt.bfloat16, tag="pr")
                    nc.scalar.activation(
                        expt[:, 0:cur], sc[:, 0:cur],
                        mybir.ActivationFunctionType.Exp,
                    )
                    for kind, key, c_ in batch:
                        if kind == 0:
                            pair_ref[key] = (expt, c_)
                        else:
                            single_ref[key] = (expt, c_)
                    batch, cur, sc = [], 0, None

                def alloc(width):
                    nonlocal cur, sc
                    if cur + width > 512:
                        flush()
                    if sc is None:
                        sc = psSc.tile([128, 512], dt.float32, tag="sc")
                    c_ = cur
                    cur += width
                    return c_

                for un in build_units(n0, n1):
                    pcol = alloc(64 * len(un))
                    nc.tensor.matmul(
                        sc[:, pcol : pcol + 64 * len(un)],
                        kt[:, 2 * plans[un[0]][0] * 64
                           : 2 * plans[un[0]][0] * 64 + 128],
                        qt[:, un[0] * 64 : (un[0] + len(un)) * 64],
                    )
                    for ui, n_ in enumerate(un):
                        batch.append((0, n_, pcol + 64 * ui))
                    for n_ in un:
                        rhs = qt[:, n_ * 64 : (n_ + 1) * 64]
                        for si, (je, jo) in enumerate(plans[n_][1]):
                            rcol = alloc(64)
                            if je is not None and jo is not None:
                                nc.tensor.matmul(
                                    sc[0:64, rcol : rcol + 64],
                                    kt[:, je * 64 : je * 64 + 64],
                                    rhs,
                                )
                                nc.tensor.matmul(
                                    sc[64:128, rcol : rcol + 64],
                                    kt[:, jo * 64 : jo * 64 + 64],
                                    rhs,
                                )
                            else:
                                # one-sided: fill BOTH halves with a
                                # 128-wide kT slice so the dead half
                                # holds bounded scores
                                ja = je if je is not None else jo - 1
                                nc.tensor.matmul(
                                    sc[:, rcol : rcol + 64],
                                    kt[:, ja * 64 : ja * 64 + 128],
                                    rhs,
                                )
                            batch.append((1, (n_, si), rcol))
                flush()
                return pair_ref, single_ref

            def build_units(n0, n1):
                """(2u, 2u+1) block pairs share the same aligned key pair
                -> merged score/ctx matmuls"""
                units = []
                n = n0
                while n <= n1:
                    if n % 2 == 0 and n + 1 <= n1:
                        units.append((n, n + 1))
                        n += 2
                    else:
                        units.append((n,))
                        n += 1
                return units

            pre_refs = {}

            # ---- stage C: projections ----
            with tc.tile_pool(name="psP", bufs=6, space="PSUM") as psP:
                def proj128(wt_sb, dst, bias):
                    for sh in range(2):
                        pts = [
                            psP.tile([128, 512], dt.float32, tag="pp",
                                     name=f"pp{i}")
                            for i in range(4)
                        ]
                        for c6 in range(NK6):
                            lhs = wt_sb[:, c6 * 128 : (c6 + 1) * 128]
                            for nb in range(4):
                                nc.tensor.matmul(
                                    pts[nb][:],
                                    lhs,
                                    XT[:, c6 * S + sh * 2048 + nb * 512
                                       : c6 * S + sh * 2048 + (nb + 1) * 512],
                                    start=(c6 == 0),
                                    stop=(c6 == NK6 - 1),
                                )
                        for nb in range(4):
                            col = sh * 2048 + nb * 512
                            nc.scalar.activation(
                                dst[:, col : col + 512],
                                pts[nb][:],
                                mybir.ActivationFunctionType.Identity,
                                bias=bias[:],
                                scale=1.0,
                            )

                proj128(WQTs, qT01, BQs)
                proj128(WKTs, kT01, BKs)
                # kz0/kz1 data halves ride the idle gpsimd queue so the
                # tiny kgz/qgz copies below (which gate dglob) stay at the
                # head of the sync queue
                nc.gpsimd.dma_start(kz0[0:64, :], kT01[0:64, :])
                nc.gpsimd.dma_start(kz1[64:128, :], kT01[64:128, :])
                for sl, j in enumerate((0, NB - 1)):
                    nc.sync.dma_start(
                        kgz0[0:64, sl * 64 : sl * 64 + 64],
                        kT01[0:64, j * 64 : (j + 1) * 64],
                    )
                    nc.sync.dma_start(
                        kgz1[64:128, sl * 64 : sl * 64 + 64],
                        kT01[64:128, j * 64 : (j + 1) * 64],
                    )
                    nc.sync.dma_start(
                        qgz0[0:64, sl * 64 : sl * 64 + 64],
                        qT01[0:64, j * 64 : (j + 1) * 64],
                    )
                    nc.sync.dma_start(
                        qgz1[64:128, sl * 64 : sl * 64 + 64],
                        qT01[64:128, j * 64 : (j + 1) * 64],
                    )

                # ---- global key scores (blocks 0,63 vs all queries) ----
                def dglob(h):
                    lhs = (kgz0, kgz1, kgz2)[h]
                    # raw q streams (the duplicated qd views in XT are not
                    # filled until after the V projection)
                    qt = qT01 if h < 2 else qk2T
                    for kb in range(8):
                        ps = psP.tile([128, 512], dt.float32, tag="pp")
                        nc.tensor.matmul(
                            ps[:], lhs[:], qt[:, kb * 512 : (kb + 1) * 512],
                        )
                        nc.scalar.activation(
                            expTg[:, h * S + kb * 512 : h * S + (kb + 1) * 512],
                            ps[:],
                            mybir.ActivationFunctionType.Exp,
                        )

                if lvl >= 2:
                    dglob(0)
                    dglob(1)

                proj128(WQK2s, qk2T, BQK2s)
                for sl, j in enumerate((0, NB - 1)):
                    nc.sync.dma_start(
                        kgz2[0:64, sl * 64 : sl * 64 + 64],
                        qk2T[64:128, j * 64 : (j + 1) * 64],
                    )
                    nc.sync.dma_start(
                        qgz2[0:64, sl * 64 : sl * 64 + 64],
                        qk2T[0:64, j * 64 : (j + 1) * 64],
                    )

                if lvl >= 2:
                    dglob(2)
                    # group-0 h0/h1 scores now: their matmuls precede the
                    # V projection in the tensor FIFO, their exps overlap
                    # V's matmuls, and ctx can start the moment V drains
                    pre_refs[(0, 0)] = emit_scores(*groups[0], 0)
                    pre_refs[(0, 1)] = emit_scores(*groups[0], 1)

                # V natural with interleaved ones columns
                for qt4 in range(8):
                    pvs = [
                        psP.tile([128, 192], dt.float32, tag="pp",
                                 name=f"pv{i}")
                        for i in range(4)
                    ]
                    for c6 in range(NK6):
                        for i in range(4):
                            t = qt4 * 4 + i
                            nc.tensor.matmul(
                                pvs[i][:],
                                XT[:, c6 * S + t * 128 : c6 * S + (t + 1) * 128],
                                WVTs[:, c6 * 192 : (c6 + 1) * 192],
                                start=(c6 == 0),
                                stop=(c6 == NK6 - 1),
                            )
                    for i in range(4):
                        t = qt4 * 4 + i
                        base = Vn[0:128, t * VW : t * VW + 64]
                        dst = AP(base.tensor, base.offset,
                                 [[NCHUNK * VW, 128], [65, 3], [1, 64]])
                        nc.vector.tensor_copy(
                            dst,
                            pvs[i][:].rearrange("p (a b) -> p a b", a=3),
                        )
                    # incremental masked-V copies (with ones columns): ride
                    # behind each qt4 chunk so VnZ is ready ~when V drains
                    c0, c1 = qt4 * 4 * VW, (qt4 + 1) * 4 * VW
                    nc.sync.dma_start(
                        VnZlo[0:64, c0:c1], Vn[0:64, c0:c1]
                    )
                    nc.scalar.dma_start(
                        VnZhi[64:128, c0:c1], Vn[64:128, c0:c1]
                    )

            # ---- post-V fills (XT tail is WAR-free from here) ----
            # Vg_glob: V rows of blocks 0 and 63 (with ones) — gates the
            # first ctx matmul of every group
            nc.sync.dma_start(Vg_glob[0:64, :], Vn[0:64, 0:VW])
            nc.scalar.dma_start(
                Vg_glob[64:128, :], Vn[64:128, 31 * VW : 32 * VW]
            )
            # kz2 = [k2; 0] (h2 runs third in each group — off-path)
            nc.gpsimd.dma_start(XT[0:64, 5 * S : 6 * S], qk2T[64:128, :])
            nc.vector.memset(XT[64:128, 5 * S : 6 * S], 0.0)

            # ---- stages D (sparse), E (global rows), G (out proj) ----
            with (
                tc.tile_pool(name="psCtx", bufs=3, space="PSUM") as psCtx,
                tc.tile_pool(name="psG", bufs=2, space="PSUM") as psG,
            ):
                def emit_norm(ctxp, h, c0, w, gap=False):
                    """Normalize ctxp [65, w] -> ctxT head h cols c0..c0+w.
                    gap=True: dst cols are blocks {0, 63} (w=128)."""
                    drow = small.tile([1, 512], dt.float32, tag="dr")
                    nc.scalar.activation(
                        drow[:, 0:w], ctxp[64:65, 0:w],
                        mybir.ActivationFunctionType.Identity,
                    )
                    rr32 = small.tile([1, 512], dt.float32, tag="rr32")
                    nc.vector.reciprocal_approx_fast(
                        rr32[:, 0:w], drow[:, 0:w]
                    )
                    rrbf = small.tile([1, 512], dt.bfloat16, tag="rr")
                    nc.vector.tensor_copy(rrbf[:, 0:w], rr32[:, 0:w])
                    rbc = psSc.tile([64, 512], dt.float32, tag="sc", name="rbc")
                    nc.tensor.matmul(
                        rbc[:, 0:w], ONES[:], rrbf[:, 0:w],
                    )
                    rbs = rbsp.tile([64, 512], dt.float32, tag="rbs")
                    nc.vector.tensor_copy(rbs[:, 0:w], rbc[:, 0:w])
                    if h == 2:
                        dstt, p0 = ctxT2, 0
                    else:
                        dstt, p0 = ctxT01, 64 * h
                    if gap:
                        if h == 1:
                            hs = hscp.tile([64, 512], dt.bfloat16, tag="hs")
                            nc.vector.tensor_mul(
                                hs[:, 0:w], ctxp[0:64, 0:w], rbs[:, 0:w]
                            )
                            base = dstt[64:128, 0:64]
                            dap = AP(base.tensor, base.offset,
                                     [[S, 64], [(NB - 1) * 64, 2], [1, 64]])
                            nc.sync.dma_start(dap, hs[:, 0:w])
                        else:
                            base = dstt[p0 : p0 + 64, 0:64]
                            dap = AP(base.tensor, base.offset,
                                     [[S, 64], [(NB - 1) * 64, 2], [1, 64]])
                            nc.vector.tensor_mul(
                                dap, ctxp[0:64, 0:w], rbs[:, 0:w]
                            )
                    else:
                        if h == 1:
                            hs = hscp.tile([64, 512], dt.bfloat16, tag="hs")
                            nc.vector.tensor_mul(
                                hs[:, 0:w], ctxp[0:64, 0:w], rbs[:, 0:w]
                            )
                            nc.sync.dma_start(
                                dstt[64:128, c0 : c0 + w], hs[:, 0:w]
                            )
                        else:
                            nc.vector.tensor_mul(
                                dstt[p0 : p0 + 64, c0 : c0 + w],
                                ctxp[0:64, 0:w], rbs[:, 0:w],
                            )

                def emit_g_chunk(t):
                    """Output projection for s-chunk t (128 rows)."""
                    pos = [
                        psG.tile([128, 384], dt.float32, tag="po",
                                 name=f"po{i}")
                        for i in range(2)
                    ]
                    for nh in range(2):
                        nc.tensor.matmul(
                            pos[nh][:],
                            ctxT01[:, t * 128 : (t + 1) * 128],
                            WOT01[:, nh * 384 : (nh + 1) * 384],
                            start=True, stop=False,
                        )
                        nc.tensor.matmul(
                            pos[nh][:],
                            ctxT2[:, t * 128 : (t + 1) * 128],
                            WOT2[:, nh * 384 : (nh + 1) * 384],
                            start=False, stop=True,
                        )
                    ot = osb.tile([128, HID], dt.float16, tag="ot")
                    nc.vector.tensor_copy(ot[:, 0:384], pos[0][:])
                    nc.vector.tensor_copy(ot[:, 384:768], pos[1][:])
                    nc.sync.dma_start(OUT_d[t * 128 : (t + 1) * 128, :], ot[:])

                norm_q = []           # pending (ctxp, h, c0, w, gap), depth 2
                next_g = 1            # next output chunk to emit (skip 0, 31)

                def emit_E():
                    # stage E: global query rows (blocks 0, 63), dense.
                    # expTE overlays XT cols 0..S per head is too big —
                    # use thirds of XT[0:3S] so all heads coexist; the
                    # three ctx chains then interleave (c outer, h inner)
                    # for better exp/matmul pipelining at the tail.
                    while norm_q:
                        emit_norm(*norm_q.pop(0))
                    for h in range(HPC):
                        kt = kzt(h)
                        qg = (qgz0, qgz1, qgz2)[h]
                        expTE = XT[:, h * S : (h + 1) * S]
                        for kb in range(8):
                            sc = psSc.tile([128, 512], dt.float32, tag="sc")
                            for i in range(4):
                                c = kb * 4 + i
                                nc.tensor.matmul(
                                    sc[:, i * 128 : (i + 1) * 128],
                                    kt[:, c * 128 : (c + 1) * 128],
                                    qg[:],
                                )
                            nc.scalar.activation(
                                expTE[:, kb * 512 : (kb + 1) * 512], sc[:],
                                mybir.ActivationFunctionType.Exp,
                            )
                    ctxes = [
                        psCtx.tile([65, 512], dt.float32, tag="cx",
                                   name=f"cxe{h}")
                        for h in range(HPC)
                    ]
                    for c in range(NCHUNK):
                        for h in range(HPC):
                            nc.tensor.matmul(
                                ctxes[h][:, 0:128],
                                Vn[:, c * VW + h * 65 : c * VW + h * 65 + 65],
                                XT[:, h * S + c * 128 : h * S + (c + 1) * 128],
                                start=(c == 0), stop=(c == NCHUNK - 1),
                            )
                    for h in range(HPC):
                        norm_q.append((ctxes[h], h, 0, 128, True))

                if lvl >= 2:
                    for gi, (n0, n1) in enumerate(groups):
                        for h in range(HPC):
                            if (gi, h) in pre_refs:
                                pair_ref, single_ref = pre_refs.pop((gi, h))
                            else:
                                pair_ref, single_ref = emit_scores(n0, n1, h)
                            units = build_units(n0, n1)

                            # lagged normalize (2 steps behind)
                            while len(norm_q) >= 2:
                                emit_norm(*norm_q.pop(0))

                            # ctx accumulation, glob-first: ONE wide N=w
                            # start matmul seeds every region's has_written
                            # bits (single start per bank — safe), singles
                            # accumulate, and each unit's merged pair matmul
                            # closes its regions with stop=True.
                            w = (n1 - n0 + 1) * 64
                            ctxp = psCtx.tile([65, 512], dt.float32, tag="cx")
                            nc.tensor.matmul(
                                ctxp[:, 0:w],
                                Vg_glob[:, h * 65 : h * 65 + 65],
                                expTg[:, h * S + n0 * 64
                                      : h * S + n0 * 64 + w],
                                start=True, stop=False,
                                skip_group_check=True,
                            )
                            for un in units:
                                for n_ in un:
                                    col = (n_ - n0) * 64
                                    for i, (je, jo) in enumerate(plans[n_][1]):
                                        set_, ecol = single_ref[(n_, i)]
                                        if je is not None:
                                            nc.tensor.matmul(
                                                ctxp[:, col : col + 64],
                                                VnZlo[:, (je // 2) * VW + h * 65
                                                      : (je // 2) * VW + h * 65 + 65],
                                                set_[:, ecol : ecol + 64],
                                                start=False, stop=False,
                                                skip_group_check=True,
                                            )
                                        if jo is not None:
                                            nc.tensor.matmul(
                                                ctxp[:, col : col + 64],
                                                VnZhi[:, (jo // 2) * VW + h * 65
                                                      : (jo // 2) * VW + h * 65 + 65],
                                                set_[:, ecol : ecol + 64],
                                                start=False, stop=False,
                                                skip_group_check=True,
                                            )
                                # closing pair matmul over the whole unit
                                # (scores allocated the unit as one
                                # contiguous region, so expt/pc are shared)
                                u = plans[un[0]][0]
                                expt, pc = pair_ref[un[0]]
                                col0 = (un[0] - n0) * 64
                                nc.tensor.matmul(
                                    ctxp[:, col0 : col0 + 64 * len(un)],
                                    Vn[:, u * VW + h * 65
                                       : u * VW + h * 65 + 65],
                                    expt[:, pc : pc + 64 * len(un)],
                                    start=False, stop=True,
                                    skip_group_check=True,
                                )
                            norm_q.append((ctxp, h, n0 * 64, w, False))

                            # sprinkle dense output-projection chunks (N=384,
                            # full-density matmuls) between heads so every
                            # HAM activity window sees high-duty work — keeps
                            # the PE clock at K=8/8 through the sparse stage.
                            # norm_q lags 2 behind: at h=0 the previous
                            # group's h2 norm is still pending, so only
                            # gi-2 is fully normalized then.
                            done_gi = gi - 2 if h == 0 else gi - 1
                            if done_gi >= 0:
                                done_n = groups[done_gi][1]
                                emitted = 0
                                while (next_g < 31 and emitted < 2
                                       and 2 * next_g + 1 <= done_n):
                                    emit_g_chunk(next_g)
                                    next_g += 1
                                    emitted += 1

                if lvl >= 3:
                    emit_E()

                while norm_q:
                    emit_norm(*norm_q.pop(0))

                # ---- stage G tail ----
                if lvl >= 4:
                    while next_g < 31:
                        emit_g_chunk(next_g)
                        next_g += 1
                    emit_g_chunk(0)
                    emit_g_chunk(31)
                else:
                    zt = osb.tile([128, HID], dt.float16, tag="ot")
                    nc.vector.memset(zt[:], 0.0)
                    for t in range(NCHUNK):
                        nc.scalar.dma_start(
                            OUT_d[t * 128 : (t + 1) * 128, :], zt[:]
                        )

                if dump:
                    nc.sync.dma_start(dmp["DQT01"].ap(), qT01[:])
                    nc.sync.dma_start(dmp["DKT01"].ap(), kT01[:])
                    nc.sync.dma_start(dmp["DQK2"].ap(), qk2T[:])
                    nc.sync.dma_start(
                        dmp["DK2LO"].ap(), XT[0:64, 5 * S : 6 * S]
                    )
                    nc.sync.dma_start(dmp["DVN"].ap(), Vn[:])
                    if lvl >= 2:
                        nc.sync.dma_start(dmp["DCTXT01"].ap(), ctxT01[:])
                        nc.sync.dma_start(dmp["DCTXT2"].ap(), ctxT2[:])

            _pools.close()

    nc.finalize()
    return nc


def kernel(X, band_mask, from_mask, to_mask, blocked_encoder_mask, rand_idx,
           Wq_w, Wq_b, Wk_w, Wk_b, Wv_w, Wv_b, Wo_w, Wo_b):
    from concourse.bass_utils import run_bass_kernel_spmd

    X = np.asarray(X, dtype=np.float32)
    rand_idx = np.asarray(rand_idx)
    Wq_w = np.asarray(Wq_w, np.float32); Wq_b = np.asarray(Wq_b, np.float32)
    Wk_w = np.asarray(Wk_w, np.float32); Wk_b = np.asarray(Wk_b, np.float32)
    Wv_w = np.asarray(Wv_w, np.float32); Wv_b = np.asarray(Wv_b, np.float32)
    Wo_w = np.asarray(Wo_w, np.float32); Wo_b = np.asarray(Wo_b, np.float32)

    blk = np.arange(NB)
    window = (blk[:, None] + np.array([-1, 0, 1])[None, :]) % NB
    glob = np.broadcast_to(np.array([0, NB - 1]), (NB, 2))
    idx = np.concatenate([window, glob, rand_idx.astype(np.int64)], axis=1)
    upto = os.environ.get("KUPTO", "G")
    key = (idx.tobytes(), upto, os.environ.get("KDUMP", "0"))
    if key not in _cache:
        _cache[key] = _build(
            tuple(tuple(int(v) for v in row) for row in idx), upto=upto
        )
    nc = _cache[key]

    sc = 1.0 / np.sqrt(HD)
    in_maps = []
    for c in range(NCORES):
        b = c // 4
        h0 = HPC * (c % 4)
        s01 = slice(h0 * HD, (h0 + 2) * HD)
        s2 = slice((h0 + 2) * HD, (h0 + 3) * HD)
        hsl = slice(h0 * HD, (h0 + HPC) * HD)
        wqk2 = np.concatenate(
            [(Wq_w[s2, :] * sc).T, Wk_w[s2, :].T], axis=1
        )  # [768, 128]
        bqk2 = np.concatenate([Wq_b[s2] * sc, Wk_b[s2]])
        in_maps.append({
            "XT": np.ascontiguousarray(X[b].T).astype(BF16),
            "WQT": np.ascontiguousarray((Wq_w[s01, :] * sc).T).astype(BF16),
            "WKT": np.ascontiguousarray(Wk_w[s01, :].T).astype(BF16),
            "WQK2": np.ascontiguousarray(wqk2).astype(BF16),
            "WVT": np.ascontiguousarray(Wv_w[hsl, :].T).astype(BF16),
            "WOT": np.ascontiguousarray(Wo_w[:, hsl].T).astype(BF16),
            "BQ": (Wq_b[s01] * sc).astype(np.float32)[:, None],
            "BK": Wk_b[s01].astype(np.float32)[:, None],
            "BQK2": bqk2.astype(np.float32)[:, None],
        })

    global _last_in_maps
    _last_in_maps = in_maps
    res = run_bass_kernel_spmd(nc, in_maps, core_ids=list(range(NCORES)))

    out = np.zeros((B, S, HID), dtype=np.float32)
    for c in range(NCORES):
        out[c // 4] += res.results[c]["OUT"].astype(np.float32)
    global _last_res
    _last_res = res
    # bias terms handled on host: Wo bias, and Wv bias pushed through Wo
    # (sum_k probs = 1, so ctx picks up Wv_b exactly).
    out += (Wo_w @ Wv_b + Wo_b)[None, None, :]
    fm = np.asarray(from_mask, np.float32).reshape(B, S)
    if not np.all(fm == 1.0):
        raise NotImplementedError("kernel assumes all-ones from_mask")
    return out

